# revision 13
# baseline (speedup 1.0000x reference)
"""Distributed GCN backbone kernel for Trainium2 (8 NeuronCores).

Strategy (per sharding hint): nodes sharded by rows across 8 cores; edges
partitioned by destination node so the segment-sum stays local; the support
table (h @ W) is all-gathered across cores each layer (halo = full table
since sources are uniform-random); per-edge features fetched with
dma_gather; segment-sum realized as PE matmuls against on-the-fly one-hot
selector matrices S[e,d] = (iota==dest_slot)*w built on the vector engine.

Dataflow per core (ROWS = N/8 local destination rows):
  GEMM1: support1 = x_shard @ W1 + b1            (PE, fp16)
  AllGather -> table1 [N, 128] fp16 (Shared DRAM)
  L1: per chunk of dest-tiles: dma_gather msgs from table1 (4 int16 windows),
      build S, psum_hT[f,d] += msgs_blk^T @ S_blk, then
      hT = relu(psum) * (dropout_mask*2)          (DVE scalar_tensor_tensor)
  GEMM2: support2 = hT^T @ W2 + b2 -> padded to 128 cols
  AllGather -> table2 [N, 128] fp16
  L2: gather msgs2 from table2, rebuild S, psum_out[d,c] += S_blk^T @ msgs2_blk
      -> out [ROWS, 64] fp32

kernel(**inputs) takes the full unsharded inputs and returns the full
[100000, 64] fp32 output.
"""

import numpy as np

import concourse.bacc as bacc
import concourse.bass as bass
import concourse.mybir as mybir
import concourse.tile as tile
from concourse.bass_utils import run_bass_kernel_spmd
from concourse.library_config import mlp

F16 = mybir.dt.float16
F32 = mybir.dt.float32
I16 = mybir.dt.int16

REAL_DIMS = dict(
    n_nodes=100000, nfeat=512, nhid=128, nclass=64,
    n_cores=8, win=32768, ch_dt=6, dropout=0.5,
)


# ----------------------------------------------------------------- host prep

def edge_structure(row, col, dims):
    """Group edges per core by (dest_tile, src_window), pad each group to a
    multiple of 128 (block) with (src=window_base, dest=0, w=0) slots.
    Block counts per (dt, win) are the max over cores so one NEFF serves all
    cores. Returns (meta, per_core_groups) where per_core_groups[c] is a list
    of (dt, win, slot_base, edge_indices) fill directives.
    """
    n_cores = dims["n_cores"]
    n = dims["n_nodes"]
    rows_pc = n // n_cores
    ndt = -(-rows_pc // 128)
    nwin = -(-n // dims["win"])
    ch = dims["ch_dt"]

    core_groups = []          # per core: {(dt,win): edge idx array}
    counts = np.zeros((n_cores, ndt, nwin), np.int64)
    for c in range(n_cores):
        sel = np.nonzero((row >= c * rows_pc) & (row < (c + 1) * rows_pc))[0]
        r_loc = row[sel] - c * rows_pc
        dt = r_loc // 128
        win = col[sel] // dims["win"]
        key = dt.astype(np.int64) * nwin + win
        order = np.argsort(key, kind="stable")
        sel, key = sel[order], key[order]
        uniq, starts = np.unique(key, return_index=True)
        starts = list(starts) + [len(sel)]
        groups = {}
        for i, k in enumerate(uniq):
            g_dt, g_w = int(k) // nwin, int(k) % nwin
            idxs = sel[starts[i]:starts[i + 1]]
            groups[(g_dt, g_w)] = idxs
            counts[c, g_dt, g_w] = len(idxs)
        core_groups.append(groups)

    B = -(-counts.max(axis=0) // 128)          # [ndt, nwin] blocks (>=0)
    B = np.maximum(B, (counts.max(axis=0) > 0).astype(np.int64))

    # chunk layout: chunks of `ch` dest-tiles; within a chunk blocks are
    # window-major: for w: for dt in chunk: B[dt][w] blocks.
    chunks = []
    block_off = 0
    for c0 in range(0, ndt, ch):
        dts = list(range(c0, min(c0 + ch, ndt)))
        woff, blk = [], 0
        for w in range(nwin):
            woff.append(blk)
            blk += int(B[dts, w].sum()) if len(dts) else 0
        # per (dt) block index lists
        dt_blocks = {}
        for dt in dts:
            lst = []
            for w in range(nwin):
                base = woff[w] + int(B[dts[0]:dt, w].sum())
                lst += list(range(base, base + int(B[dt, w])))
            dt_blocks[dt] = lst
        nb_win = [int(B[dts, w].sum()) for w in range(nwin)]
        chunks.append(dict(dts=dts, nb_win=nb_win, nbc=blk,
                           block_off=block_off, dt_blocks=dt_blocks, woff=woff))
        block_off += blk
    nb_tot = block_off

    meta = dict(B=B, chunks=chunks, nb_tot=nb_tot, ndt=ndt, nwin=nwin,
                rows_pc=rows_pc)
    return meta, core_groups


def fill_edge_arrays(meta, groups, row, col, ew, core, dims):
    """Build per-core (idx_wrapped[i16 128 x S/16], dest[f16 128 x NB],
    w[f16 128 x NB]) slot arrays in the chunk/window-major block layout."""
    nwin = meta["nwin"]
    rows_pc = meta["rows_pc"]
    nslots = meta["nb_tot"] * 128
    idx_f = np.zeros(nslots, np.int32)
    dest_f = np.zeros(nslots, np.float16)
    w_f = np.zeros(nslots, np.float16)
    for chk in meta["chunks"]:
        for dt in chk["dts"]:
            blocks = chk["dt_blocks"][dt]
            # window sub-ranges of this dt's block list
            bpos = 0
            for w in range(nwin):
                nb = int(meta["B"][dt, w])
                if nb == 0:
                    continue
                base_blk = blocks[bpos]
                bpos += nb
                base = (chk["block_off"] + base_blk) * 128
                e = groups.get((dt, w))
                if e is None:
                    idx_f[base:base + nb * 128] = 0  # window-local 0
                    continue
                cnt = len(e)
                idx_f[base:base + cnt] = col[e] - w * dims["win"]
                dest_f[base:base + cnt] = (row[e] - core * rows_pc - dt * 128
                                           ).astype(np.float16)
                w_f[base:base + cnt] = ew[e].astype(np.float16)
                # padding slots already 0 (window-local idx 0, w=0)
    assert (idx_f >= 0).all() and (idx_f < 32768).all()
    idx16 = idx_f.astype(np.int16).reshape(-1, 16).T          # [16, S/16]
    idx16 = np.tile(idx16, (8, 1))                            # [128, S/16]
    # edw[p, 0, b] = dest_slot, edw[p, 1, b] = weight (f32: DVE scalar APs)
    edw = np.empty((128, 2, meta["nb_tot"]), np.float32)
    edw[:, 0, :] = dest_f.astype(np.float32).reshape(-1, 128).T
    edw[:, 1, :] = w_f.astype(np.float32).reshape(-1, 128).T
    return idx16, edw


# ------------------------------------------------------------ kernel builder

def build_nc(meta, dims, debug=False):
    n = dims["n_nodes"]
    nfeat, nhid, ncls = dims["nfeat"], dims["nhid"], dims["nclass"]
    rows = meta["rows_pc"]
    nwin = meta["nwin"]
    win = dims["win"]
    nb_tot = meta["nb_tot"]
    nslots = nb_tot * 128
    ndt = meta["ndt"]
    nrt = -(-rows // 128)
    nkt = nfeat // 128
    t2w = 128  # layer-2 table width (nclass padded so gather elem = 256B)
    assert nhid == 128

    nc = bacc.Bacc("TRN2", target_bir_lowering=False, debug=debug)

    xT_d = nc.dram_tensor("xTk", [128, nkt, rows], F16, kind="ExternalInput")
    mask_d = nc.dram_tensor("maskT2", [nhid, rows], F16, kind="ExternalInput")
    w1_d = nc.dram_tensor("W1", [nfeat, nhid], F16, kind="ExternalInput")
    b1_d = nc.dram_tensor("b1row", [1, nhid], F16, kind="ExternalInput")
    w2_d = nc.dram_tensor("W2", [nhid, ncls], F16, kind="ExternalInput")
    b2_d = nc.dram_tensor("b2row", [1, ncls], F16, kind="ExternalInput")
    eidx_d = nc.dram_tensor("eidx", [128, nslots // 16], I16, kind="ExternalInput")
    edw_d = nc.dram_tensor("edw", [128, 2, nb_tot], F32, kind="ExternalInput")
    out_d = nc.dram_tensor("out", [rows, ncls], F32, kind="ExternalOutput")

    rg = [list(range(dims["n_cores"]))]

    with tile.TileContext(nc) as tc:
        with (
            tc.tile_pool(name="dram", bufs=1, space="DRAM") as dpool,
            tc.tile_pool(name="const", bufs=1) as cpool,
            tc.tile_pool(name="persist", bufs=1) as hpool,
            tc.tile_pool(name="stream", bufs=3) as spool,
            tc.tile_pool(name="chunkio", bufs=2) as iopool,
            tc.tile_pool(name="big", bufs=2) as bigpool,
            tc.tile_pool(name="psum", bufs=4, space="PSUM") as ppool,
        ):
            nc.gpsimd.load_library(mlp)

            sup1_b = dpool.tile([rows, nhid], F16, name="sup1_b")
            table1 = dpool.tile([n, nhid], F16, addr_space="Shared", name="table1")
            sup2_b = dpool.tile([rows, t2w], F16, name="sup2_b")
            table2 = dpool.tile([n, t2w], F16, addr_space="Shared", name="table2")

            # constants
            w1_t = cpool.tile([128, nkt, nhid], F16, name="w1kt")
            for kt in range(nkt):
                nc.sync.dma_start(w1_t[:, kt, :], w1_d[kt * 128:(kt + 1) * 128, :])
            b1_t = cpool.tile([1, nhid], F16, name="b1_t")
            nc.sync.dma_start(b1_t[:], b1_d[:, :])
            w2_t = cpool.tile([128, ncls], F16, name="w2_t")
            nc.sync.dma_start(w2_t[:], w2_d[:, :])
            b2_t = cpool.tile([1, ncls], F16, name="b2_t")
            nc.sync.dma_start(b2_t[:], b2_d[:, :])
            ones_t = cpool.tile([1, 128], F16, name="ones_t")
            nc.vector.memset(ones_t[:], 1.0)
            iota_t = cpool.tile([128, 128], F16, name="iota_t")
            nc.gpsimd.iota(iota_t[:], pattern=[[1, 128]], base=0,
                           channel_multiplier=0,
                           allow_small_or_imprecise_dtypes=True)
            hT = hpool.tile([nhid, rows], F16, name="hT")

            # ---------------- GEMM1: support1 = x @ W1 + b1 -> table1 shard
            for rt in range(nrt):
                r0 = rt * 128
                rc = min(128, rows - r0)
                xk = spool.tile([128, nkt, 128], F16, name="xk")
                nc.sync.dma_start(xk[:, :, :rc], xT_d[:, :, r0:r0 + rc])
                ps = ppool.tile([128, nhid], F32, name="ps", tag="ps")
                nc.tensor.matmul(ps[:rc, :], ones_t[:1, :rc], b1_t[:1, :],
                                 start=True, stop=False)
                for kt in range(nkt):
                    nc.tensor.matmul(ps[:rc, :], xk[:, kt, :rc], w1_t[:, kt, :],
                                     start=False, stop=(kt == nkt - 1))
                sup = spool.tile([128, nhid], F16, name="sup")
                nc.scalar.activation(sup[:rc, :], ps[:rc, :],
                                     mybir.ActivationFunctionType.Copy)
                nc.sync.dma_start(sup1_b[r0:r0 + rc, :], sup[:rc, :])

            nc.gpsimd.collective_compute(
                "AllGather", mybir.AluOpType.bypass, replica_groups=rg,
                ins=[sup1_b[:].opt()], outs=[table1[:].opt()])

            # ---------------- layer loops (shared structure)
            def edge_layer(table, layer):
                for chk in meta["chunks"]:
                    nbc = chk["nbc"]
                    if nbc == 0:
                        continue
                    boff = chk["block_off"]
                    msgs = bigpool.tile([128, nbc, 128], F16, name="msgs",
                                        tag="msgs")
                    s_t = bigpool.tile([128, nbc, 128], F16, name="s_t",
                                       tag="s_t")
                    idx_t = iopool.tile([128, nbc * 8], I16, name="idx_t",
                                        tag="idx_t")
                    edw_t = iopool.tile([128, 2, nbc], F32, name="edw_t",
                                        tag="edw_t")
                    nc.sync.dma_start(idx_t[:],
                                      eidx_d[:, boff * 8:(boff + nbc) * 8])
                    nc.sync.dma_start(edw_t[:], edw_d[:, :, boff:boff + nbc])

                    wb = 0
                    for w in range(nwin):
                        nbw = chk["nb_win"][w]
                        lo = w * win
                        hi = min(n, lo + win)
                        for g0 in range(0, nbw, 8):
                            gb = min(8, nbw - g0)
                            nc.gpsimd.dma_gather(
                                msgs[:, wb + g0:wb + g0 + gb, :],
                                table[lo:hi, :],
                                idx_t[:, (wb + g0) * 8:(wb + g0 + gb) * 8],
                                gb * 128, gb * 128, 128)
                        wb += nbw

                    for b in range(nbc):
                        nc.vector.tensor_scalar(
                            s_t[:, b, :], iota_t[:],
                            edw_t[:, 0, b:b + 1], edw_t[:, 1, b:b + 1],
                            mybir.AluOpType.is_equal, mybir.AluOpType.mult)

                    if layer == 1:
                        mk = iopool.tile([128, len(chk["dts"]) * 128], F16,
                                         name="mk", tag="mk")
                        m0 = chk["dts"][0] * 128
                        mw = min(rows, (chk["dts"][-1] + 1) * 128) - m0
                        nc.sync.dma_start(mk[:, :mw], mask_d[:, m0:m0 + mw])

                    for di, dt in enumerate(chk["dts"]):
                        blocks = chk["dt_blocks"][dt]
                        if not blocks:
                            continue
                        d0 = dt * 128
                        dc = min(128, rows - d0)
                        if layer == 1:
                            ph = ppool.tile([128, 128], F32, name="ph", tag="ps")
                            for i, b in enumerate(blocks):
                                nc.tensor.matmul(ph[:, :dc], msgs[:, b, :],
                                                 s_t[:, b, :dc],
                                                 start=(i == 0),
                                                 stop=(i == len(blocks) - 1))
                            nc.vector.scalar_tensor_tensor(
                                hT[:, d0:d0 + dc], ph[:, :dc], 0.0,
                                mk[:, di * 128:di * 128 + dc],
                                op0=mybir.AluOpType.max,
                                op1=mybir.AluOpType.mult)
                            # GEMM2 for this row-tile immediately (hides the
                            # layer-1 -> AllGather2 join under the L1 gathers)
                            ps2 = ppool.tile([128, ncls], F32, name="ps2",
                                             tag="ps")
                            nc.tensor.matmul(ps2[:dc, :], ones_t[:1, :dc],
                                             b2_t[:1, :], start=True, stop=False)
                            nc.tensor.matmul(ps2[:dc, :], hT[:, d0:d0 + dc],
                                             w2_t[:, :], start=False, stop=True)
                            stg = spool.tile([128, t2w], F16, name="stg")
                            nc.vector.memset(stg[:, ncls:], 0.0)
                            nc.scalar.activation(
                                stg[:dc, :ncls], ps2[:dc, :],
                                mybir.ActivationFunctionType.Copy)
                            nc.sync.dma_start(sup2_b[d0:d0 + dc, :],
                                              stg[:dc, :])
                        else:
                            po = ppool.tile([128, ncls], F32, name="po", tag="ps")
                            for i, b in enumerate(blocks):
                                nc.tensor.matmul(po[:dc, :], s_t[:, b, :dc],
                                                 msgs[:, b, :ncls],
                                                 start=(i == 0),
                                                 stop=(i == len(blocks) - 1))
                            ot = spool.tile([128, ncls], F32, name="ot")
                            nc.scalar.activation(
                                ot[:dc, :], po[:dc, :],
                                mybir.ActivationFunctionType.Copy)
                            nc.sync.dma_start(out_d[d0:d0 + dc, :], ot[:dc, :])

            edge_layer(table1, 1)

            nc.gpsimd.collective_compute(
                "AllGather", mybir.AluOpType.bypass, replica_groups=rg,
                ins=[sup2_b[:].opt()], outs=[table2[:].opt()])

            edge_layer(table2, 2)

    nc.compile()
    return nc


# ----------------------------------------------------------------- run logic

_CACHE = {}


def _get_mask2(n, nhid, dropout):
    key = (n, nhid, dropout)
    if key not in _CACHE:
        import jax
        with jax.default_device(jax.devices("cpu")[0]):
            keep = 1.0 - dropout
            m = jax.random.bernoulli(jax.random.key(42), keep, (n, nhid))
            _CACHE[key] = (np.asarray(m).astype(np.float32) / keep)
    return _CACHE[key]


def prepare(inputs, dims):
    x = inputs["x"]; ew = inputs["edge_weight"]
    W1 = inputs["W1"]; b1 = inputs["b1"]; W2 = inputs["W2"]; b2 = inputs["b2"]
    row = inputs["row"]; col = inputs["col"]
    n_cores = dims["n_cores"]
    meta, groups = edge_structure(row, col, dims)
    rows = meta["rows_pc"]
    mask2 = _get_mask2(dims["n_nodes"], dims["nhid"], dims["dropout"])
    in_maps = []
    for c in range(n_cores):
        idx16, edw = fill_edge_arrays(meta, groups[c], row, col, ew, c, dims)
        xT = x[c * rows:(c + 1) * rows].T.astype(np.float16)   # [nfeat, rows]
        nkt = dims["nfeat"] // 128
        xTk = np.ascontiguousarray(
            xT.reshape(nkt, 128, rows).transpose(1, 0, 2))     # [128, nkt, rows]
        in_maps.append(dict(
            xTk=xTk,
            maskT2=np.ascontiguousarray(mask2[c * rows:(c + 1) * rows].T
                                        ).astype(np.float16),
            W1=W1.astype(np.float16),
            b1row=b1.reshape(1, -1).astype(np.float16),
            W2=W2.astype(np.float16),
            b2row=b2.reshape(1, -1).astype(np.float16),
            eidx=idx16, edw=edw,
        ))
    return meta, in_maps


def run(inputs, dims, trace=False):
    meta, in_maps = prepare(inputs, dims)
    key = ("nc", tuple(meta["B"].ravel()), tuple(sorted(dims.items())))
    if key not in _CACHE:
        _CACHE[key] = build_nc(meta, dims)
    nc = _CACHE[key]
    res = run_bass_kernel_spmd(nc, in_maps,
                               core_ids=list(range(dims["n_cores"])),
                               trace=trace)
    out = np.concatenate([r["out"] for r in res.results], axis=0)
    return out, res


def kernel(**inputs):
    out, _ = run(inputs, REAL_DIMS)
    return out


# revision 14
# speedup vs baseline: 1.0728x; 1.0728x over previous
"""Distributed GCN backbone kernel for Trainium2 (8 NeuronCores).

Strategy (per sharding hint): nodes sharded by rows across 8 cores; edges
partitioned by destination node so the segment-sum stays local; the support
table (h @ W) is all-gathered across cores each layer (halo = full table
since sources are uniform-random); per-edge features fetched with
dma_gather; segment-sum realized as PE matmuls against on-the-fly one-hot
selector matrices S[e,d] = (iota==dest_slot)*w built on the vector engine.

Dataflow per core (ROWS = N/8 local destination rows):
  GEMM1: support1 = x_shard @ W1 + b1            (PE, fp16)
  AllGather -> table1 [N, 128] fp16 (Shared DRAM)
  L1: per chunk of dest-tiles: dma_gather msgs from table1 (4 int16 windows),
      build S, psum_hT[f,d] += msgs_blk^T @ S_blk, then
      hT = relu(psum) * (dropout_mask*2)          (DVE scalar_tensor_tensor)
  GEMM2: support2 = hT^T @ W2 + b2 -> padded to 128 cols
  AllGather -> table2 [N, 128] fp16
  L2: gather msgs2 from table2, rebuild S, psum_out[d,c] += S_blk^T @ msgs2_blk
      -> out [ROWS, 64] fp32

kernel(**inputs) takes the full unsharded inputs and returns the full
[100000, 64] fp32 output.
"""

import numpy as np

import concourse.bacc as bacc
import concourse.bass as bass
import concourse.mybir as mybir
import concourse.tile as tile
from concourse.bass_utils import run_bass_kernel_spmd
from concourse.library_config import mlp

F16 = mybir.dt.float16
F32 = mybir.dt.float32
I16 = mybir.dt.int16

REAL_DIMS = dict(
    n_nodes=100000, nfeat=512, nhid=128, nclass=64,
    n_cores=8, win=32768, ch_dt=6, dropout=0.5,
)


# ----------------------------------------------------------------- host prep

def edge_structure(row, col, dims):
    """Group edges per core by (dest_tile, src_window), pad each group to a
    multiple of 128 (block) with (src=window_base, dest=0, w=0) slots.
    Block counts per (dt, win) are the max over cores so one NEFF serves all
    cores. Returns (meta, per_core_groups) where per_core_groups[c] is a list
    of (dt, win, slot_base, edge_indices) fill directives.
    """
    n_cores = dims["n_cores"]
    n = dims["n_nodes"]
    rows_pc = n // n_cores
    ndt = -(-rows_pc // 128)
    nwin = -(-n // dims["win"])
    ch = dims["ch_dt"]

    core_groups = []          # per core: {(dt,win): edge idx array}
    counts = np.zeros((n_cores, ndt, nwin), np.int64)
    for c in range(n_cores):
        sel = np.nonzero((row >= c * rows_pc) & (row < (c + 1) * rows_pc))[0]
        r_loc = row[sel] - c * rows_pc
        dt = r_loc // 128
        win = col[sel] // dims["win"]
        key = dt.astype(np.int64) * nwin + win
        order = np.argsort(key, kind="stable")
        sel, key = sel[order], key[order]
        uniq, starts = np.unique(key, return_index=True)
        starts = list(starts) + [len(sel)]
        groups = {}
        for i, k in enumerate(uniq):
            g_dt, g_w = int(k) // nwin, int(k) % nwin
            idxs = sel[starts[i]:starts[i + 1]]
            groups[(g_dt, g_w)] = idxs
            counts[c, g_dt, g_w] = len(idxs)
        core_groups.append(groups)

    B = -(-counts.max(axis=0) // 128)          # [ndt, nwin] blocks (>=0)
    B = np.maximum(B, (counts.max(axis=0) > 0).astype(np.int64))

    # chunk layout: chunks of `ch` dest-tiles; within a chunk blocks are
    # window-major: for w: for dt in chunk: B[dt][w] blocks.
    chunks = []
    block_off = 0
    for c0 in range(0, ndt, ch):
        dts = list(range(c0, min(c0 + ch, ndt)))
        woff, blk = [], 0
        for w in range(nwin):
            woff.append(blk)
            blk += int(B[dts, w].sum()) if len(dts) else 0
        # per (dt) block index lists
        dt_blocks = {}
        for dt in dts:
            lst = []
            for w in range(nwin):
                base = woff[w] + int(B[dts[0]:dt, w].sum())
                lst += list(range(base, base + int(B[dt, w])))
            dt_blocks[dt] = lst
        nb_win = [int(B[dts, w].sum()) for w in range(nwin)]
        chunks.append(dict(dts=dts, nb_win=nb_win, nbc=blk,
                           block_off=block_off, dt_blocks=dt_blocks, woff=woff))
        block_off += blk
    nb_tot = block_off

    meta = dict(B=B, chunks=chunks, nb_tot=nb_tot, ndt=ndt, nwin=nwin,
                rows_pc=rows_pc)
    return meta, core_groups


def fill_edge_arrays(meta, groups, row, col, ew, core, dims):
    """Build per-core (idx_wrapped[i16 128 x S/16], dest[f16 128 x NB],
    w[f16 128 x NB]) slot arrays in the chunk/window-major block layout."""
    nwin = meta["nwin"]
    rows_pc = meta["rows_pc"]
    nslots = meta["nb_tot"] * 128
    idx_f = np.zeros(nslots, np.int32)
    dest_f = np.zeros(nslots, np.float16)
    w_f = np.zeros(nslots, np.float16)
    for chk in meta["chunks"]:
        for dt in chk["dts"]:
            blocks = chk["dt_blocks"][dt]
            # window sub-ranges of this dt's block list
            bpos = 0
            for w in range(nwin):
                nb = int(meta["B"][dt, w])
                if nb == 0:
                    continue
                base_blk = blocks[bpos]
                bpos += nb
                base = (chk["block_off"] + base_blk) * 128
                e = groups.get((dt, w))
                if e is None:
                    idx_f[base:base + nb * 128] = 0  # window-local 0
                    continue
                cnt = len(e)
                idx_f[base:base + cnt] = col[e] - w * dims["win"]
                dest_f[base:base + cnt] = (row[e] - core * rows_pc - dt * 128
                                           ).astype(np.float16)
                w_f[base:base + cnt] = ew[e].astype(np.float16)
                # padding slots already 0 (window-local idx 0, w=0)
    assert (idx_f >= 0).all() and (idx_f < 32768).all()
    idx16 = idx_f.astype(np.int16).reshape(-1, 16).T          # [16, S/16]
    idx16 = np.tile(idx16, (8, 1))                            # [128, S/16]
    # edw[p, 0, b] = dest_slot, edw[p, 1, b] = weight (f32: DVE scalar APs)
    edw = np.empty((128, 2, meta["nb_tot"]), np.float32)
    edw[:, 0, :] = dest_f.astype(np.float32).reshape(-1, 128).T
    edw[:, 1, :] = w_f.astype(np.float32).reshape(-1, 128).T
    return idx16, edw


# ------------------------------------------------------------ kernel builder

def build_nc(meta, dims, debug=False):
    n = dims["n_nodes"]
    nfeat, nhid, ncls = dims["nfeat"], dims["nhid"], dims["nclass"]
    rows = meta["rows_pc"]
    nwin = meta["nwin"]
    win = dims["win"]
    nb_tot = meta["nb_tot"]
    nslots = nb_tot * 128
    ndt = meta["ndt"]
    nrt = -(-rows // 128)
    nkt = nfeat // 128
    t2w = 128  # layer-2 table width (nclass padded so gather elem = 256B)
    assert nhid == 128

    nc = bacc.Bacc("TRN2", target_bir_lowering=False, debug=debug)

    xT_d = nc.dram_tensor("xTk", [128, nkt, rows], F16, kind="ExternalInput")
    mask_d = nc.dram_tensor("maskT2", [nhid, rows], F16, kind="ExternalInput")
    w1_d = nc.dram_tensor("W1", [nfeat, nhid], F16, kind="ExternalInput")
    b1_d = nc.dram_tensor("b1row", [1, nhid], F16, kind="ExternalInput")
    w2_d = nc.dram_tensor("W2", [nhid, ncls], F16, kind="ExternalInput")
    b2_d = nc.dram_tensor("b2row", [1, ncls], F16, kind="ExternalInput")
    eidx_d = nc.dram_tensor("eidx", [128, nslots // 16], I16, kind="ExternalInput")
    edw_d = nc.dram_tensor("edw", [128, 2, nb_tot], F32, kind="ExternalInput")
    out_d = nc.dram_tensor("out", [rows, ncls], F32, kind="ExternalOutput")

    rg = [list(range(dims["n_cores"]))]

    with tile.TileContext(nc) as tc:
        with (
            tc.tile_pool(name="dram", bufs=1, space="DRAM") as dpool,
            tc.tile_pool(name="const", bufs=1) as cpool,
            tc.tile_pool(name="persist", bufs=1) as hpool,
            tc.tile_pool(name="stream", bufs=3) as spool,
            tc.tile_pool(name="chunkio", bufs=2) as iopool,
            tc.tile_pool(name="big", bufs=2) as bigpool,
            tc.tile_pool(name="psum", bufs=4, space="PSUM") as ppool,
            tc.tile_pool(name="psum2", bufs=2, space="PSUM") as ppool2,
        ):
            nc.gpsimd.load_library(mlp)

            sup1_b = dpool.tile([rows, nhid], F16, name="sup1_b")
            table1 = dpool.tile([n, nhid], F16, addr_space="Shared", name="table1")
            sup2_b = dpool.tile([rows, t2w], F16, name="sup2_b")
            table2 = dpool.tile([n, t2w], F16, addr_space="Shared", name="table2")

            # constants
            w1_t = cpool.tile([128, nkt, nhid], F16, name="w1kt")
            for kt in range(nkt):
                nc.sync.dma_start(w1_t[:, kt, :], w1_d[kt * 128:(kt + 1) * 128, :])
            b1_t = cpool.tile([1, nhid], F16, name="b1_t")
            nc.sync.dma_start(b1_t[:], b1_d[:, :])
            w2_t = cpool.tile([128, ncls], F16, name="w2_t")
            nc.sync.dma_start(w2_t[:], w2_d[:, :])
            b2_t = cpool.tile([1, ncls], F16, name="b2_t")
            nc.sync.dma_start(b2_t[:], b2_d[:, :])
            ones_t = cpool.tile([1, 128], F16, name="ones_t")
            nc.vector.memset(ones_t[:], 1.0)
            iota_t = cpool.tile([128, 128], F16, name="iota_t")
            nc.gpsimd.iota(iota_t[:], pattern=[[1, 128]], base=0,
                           channel_multiplier=0,
                           allow_small_or_imprecise_dtypes=True)
            hT = hpool.tile([nhid, rows], F16, name="hT")

            # ---------------- GEMM1: support1 = x @ W1 + b1 -> table1 shard
            for rt in range(nrt):
                r0 = rt * 128
                rc = min(128, rows - r0)
                xk = spool.tile([128, nkt, 128], F16, name="xk")
                nc.sync.dma_start(xk[:, :, :rc], xT_d[:, :, r0:r0 + rc])
                ps = ppool.tile([128, nhid], F32, name="ps", tag="ps")
                nc.tensor.matmul(ps[:rc, :], ones_t[:1, :rc], b1_t[:1, :],
                                 start=True, stop=False)
                for kt in range(nkt):
                    nc.tensor.matmul(ps[:rc, :], xk[:, kt, :rc], w1_t[:, kt, :],
                                     start=False, stop=(kt == nkt - 1))
                sup = spool.tile([128, nhid], F16, name="sup")
                nc.scalar.activation(sup[:rc, :], ps[:rc, :],
                                     mybir.ActivationFunctionType.Copy)
                nc.sync.dma_start(sup1_b[r0:r0 + rc, :], sup[:rc, :])

            nc.gpsimd.collective_compute(
                "AllGather", mybir.AluOpType.bypass, replica_groups=rg,
                ins=[sup1_b[:].opt()], outs=[table1[:].opt()])

            # ---------------- layer loops (shared structure)
            def edge_layer(table, layer):
                for chk in meta["chunks"]:
                    nbc = chk["nbc"]
                    if nbc == 0:
                        continue
                    boff = chk["block_off"]
                    msgs = bigpool.tile([128, nbc, 128], F16, name="msgs",
                                        tag="msgs")
                    s_t = bigpool.tile([128, nbc, 128], F16, name="s_t",
                                       tag="s_t")
                    idx_t = iopool.tile([128, nbc * 8], I16, name="idx_t",
                                        tag="idx_t")
                    edw_t = iopool.tile([128, 2, nbc], F32, name="edw_t",
                                        tag="edw_t")
                    nc.sync.dma_start(idx_t[:],
                                      eidx_d[:, boff * 8:(boff + nbc) * 8])
                    nc.sync.dma_start(edw_t[:], edw_d[:, :, boff:boff + nbc])

                    wb = 0
                    for w in range(nwin):
                        nbw = chk["nb_win"][w]
                        lo = w * win
                        hi = min(n, lo + win)
                        for g0 in range(0, nbw, 8):
                            gb = min(8, nbw - g0)
                            nc.gpsimd.dma_gather(
                                msgs[:, wb + g0:wb + g0 + gb, :],
                                table[lo:hi, :],
                                idx_t[:, (wb + g0) * 8:(wb + g0 + gb) * 8],
                                gb * 128, gb * 128, 128)
                        wb += nbw

                    for b in range(nbc):
                        nc.vector.tensor_scalar(
                            s_t[:, b, :], iota_t[:],
                            edw_t[:, 0, b:b + 1], edw_t[:, 1, b:b + 1],
                            mybir.AluOpType.is_equal, mybir.AluOpType.mult)

                    if layer == 1:
                        mk = iopool.tile([128, len(chk["dts"]) * 128], F16,
                                         name="mk", tag="mk")
                        m0 = chk["dts"][0] * 128
                        mw = min(rows, (chk["dts"][-1] + 1) * 128) - m0
                        nc.sync.dma_start(mk[:, :mw], mask_d[:, m0:m0 + mw])

                    for di, dt in enumerate(chk["dts"]):
                        blocks = chk["dt_blocks"][dt]
                        if not blocks:
                            continue
                        d0 = dt * 128
                        dc = min(128, rows - d0)
                        if layer == 1:
                            ph = ppool.tile([128, 128], F32, name="ph", tag="ps")
                            for i, b in enumerate(blocks):
                                nc.tensor.matmul(ph[:, :dc], msgs[:, b, :],
                                                 s_t[:, b, :dc],
                                                 start=(i == 0),
                                                 stop=(i == len(blocks) - 1))
                            nc.vector.scalar_tensor_tensor(
                                hT[:, d0:d0 + dc], ph[:, :dc], 0.0,
                                mk[:, di * 128:di * 128 + dc],
                                op0=mybir.AluOpType.max,
                                op1=mybir.AluOpType.mult)
                            # GEMM2 for this row-tile immediately (hides the
                            # layer-1 -> AllGather2 join under the L1 gathers)
                            ps2 = ppool2.tile([128, ncls], F32, name="ps2",
                                              tag="ps2")
                            nc.tensor.matmul(ps2[:dc, :], ones_t[:1, :dc],
                                             b2_t[:1, :], start=True, stop=False)
                            nc.tensor.matmul(ps2[:dc, :], hT[:, d0:d0 + dc],
                                             w2_t[:, :], start=False, stop=True)
                            stg = spool.tile([128, t2w], F16, name="stg")
                            nc.vector.memset(stg[:, ncls:], 0.0)
                            nc.scalar.activation(
                                stg[:dc, :ncls], ps2[:dc, :],
                                mybir.ActivationFunctionType.Copy)
                            nc.sync.dma_start(sup2_b[d0:d0 + dc, :],
                                              stg[:dc, :])
                        else:
                            po = ppool.tile([128, ncls], F32, name="po", tag="ps")
                            for i, b in enumerate(blocks):
                                nc.tensor.matmul(po[:dc, :], s_t[:, b, :dc],
                                                 msgs[:, b, :ncls],
                                                 start=(i == 0),
                                                 stop=(i == len(blocks) - 1))
                            ot = spool.tile([128, ncls], F32, name="ot")
                            nc.scalar.activation(
                                ot[:dc, :], po[:dc, :],
                                mybir.ActivationFunctionType.Copy)
                            nc.sync.dma_start(out_d[d0:d0 + dc, :], ot[:dc, :])

            edge_layer(table1, 1)

            nc.gpsimd.collective_compute(
                "AllGather", mybir.AluOpType.bypass, replica_groups=rg,
                ins=[sup2_b[:].opt()], outs=[table2[:].opt()])

            edge_layer(table2, 2)

    nc.compile()
    return nc


# ----------------------------------------------------------------- run logic

_CACHE = {}


def _get_mask2(n, nhid, dropout):
    key = (n, nhid, dropout)
    if key not in _CACHE:
        import jax
        with jax.default_device(jax.devices("cpu")[0]):
            keep = 1.0 - dropout
            m = jax.random.bernoulli(jax.random.key(42), keep, (n, nhid))
            _CACHE[key] = (np.asarray(m).astype(np.float32) / keep)
    return _CACHE[key]


def prepare(inputs, dims):
    x = inputs["x"]; ew = inputs["edge_weight"]
    W1 = inputs["W1"]; b1 = inputs["b1"]; W2 = inputs["W2"]; b2 = inputs["b2"]
    row = inputs["row"]; col = inputs["col"]
    n_cores = dims["n_cores"]
    meta, groups = edge_structure(row, col, dims)
    rows = meta["rows_pc"]
    mask2 = _get_mask2(dims["n_nodes"], dims["nhid"], dims["dropout"])
    in_maps = []
    for c in range(n_cores):
        idx16, edw = fill_edge_arrays(meta, groups[c], row, col, ew, c, dims)
        xT = x[c * rows:(c + 1) * rows].T.astype(np.float16)   # [nfeat, rows]
        nkt = dims["nfeat"] // 128
        xTk = np.ascontiguousarray(
            xT.reshape(nkt, 128, rows).transpose(1, 0, 2))     # [128, nkt, rows]
        in_maps.append(dict(
            xTk=xTk,
            maskT2=np.ascontiguousarray(mask2[c * rows:(c + 1) * rows].T
                                        ).astype(np.float16),
            W1=W1.astype(np.float16),
            b1row=b1.reshape(1, -1).astype(np.float16),
            W2=W2.astype(np.float16),
            b2row=b2.reshape(1, -1).astype(np.float16),
            eidx=idx16, edw=edw,
        ))
    return meta, in_maps


def run(inputs, dims, trace=False):
    meta, in_maps = prepare(inputs, dims)
    key = ("nc", tuple(meta["B"].ravel()), tuple(sorted(dims.items())))
    if key not in _CACHE:
        _CACHE[key] = build_nc(meta, dims)
    nc = _CACHE[key]
    res = run_bass_kernel_spmd(nc, in_maps,
                               core_ids=list(range(dims["n_cores"])),
                               trace=trace)
    out = np.concatenate([r["out"] for r in res.results], axis=0)
    return out, res


def kernel(**inputs):
    out, _ = run(inputs, REAL_DIMS)
    return out


# revision 16
# speedup vs baseline: 1.1118x; 1.0364x over previous
"""Distributed GCN backbone kernel for Trainium2 (8 NeuronCores).

Strategy (per sharding hint): nodes sharded by rows across 8 cores; edges
partitioned by destination node so the segment-sum stays local; the support
table (h @ W) is all-gathered across cores each layer (halo = full table
since sources are uniform-random); per-edge features fetched with
dma_gather; segment-sum realized as PE matmuls against on-the-fly one-hot
selector matrices S[e,d] = (iota==dest_slot)*w built on the vector engine.

Dataflow per core (ROWS = N/8 local destination rows):
  GEMM1: support1 = x_shard @ W1 + b1            (PE, fp16)
  AllGather -> table1 [N, 128] fp16 (Shared DRAM)
  L1: per chunk of dest-tiles: dma_gather msgs from table1 (4 int16 windows),
      build S, psum_hT[f,d] += msgs_blk^T @ S_blk, then
      hT = relu(psum) * (dropout_mask*2)          (DVE scalar_tensor_tensor)
  GEMM2: support2 = hT^T @ W2 + b2 -> padded to 128 cols
  AllGather -> table2 [N, 128] fp16
  L2: gather msgs2 from table2, rebuild S, psum_out[d,c] += S_blk^T @ msgs2_blk
      -> out [ROWS, 64] fp32

kernel(**inputs) takes the full unsharded inputs and returns the full
[100000, 64] fp32 output.
"""

import numpy as np

import concourse.bacc as bacc
import concourse.bass as bass
import concourse.mybir as mybir
import concourse.tile as tile
from concourse.bass_utils import run_bass_kernel_spmd
from concourse.library_config import mlp

F16 = mybir.dt.float16
F32 = mybir.dt.float32
I16 = mybir.dt.int16

REAL_DIMS = dict(
    n_nodes=100000, nfeat=512, nhid=128, nclass=64,
    n_cores=8, win=32768, ch_dt=6, dropout=0.5, split_a=4096,
)


# ----------------------------------------------------------------- host prep

def edge_structure(row, col, dims):
    """Group edges per core by (dest_tile, src_window), pad each group to a
    multiple of 128 (block) with (src=window_base, dest=0, w=0) slots.
    Block counts per (dt, win) are the max over cores so one NEFF serves all
    cores. Returns (meta, per_core_groups) where per_core_groups[c] is a list
    of (dt, win, slot_base, edge_indices) fill directives.
    """
    n_cores = dims["n_cores"]
    n = dims["n_nodes"]
    rows_pc = n // n_cores
    ndt = -(-rows_pc // 128)
    nwin = -(-n // dims["win"])
    ch = dims["ch_dt"]

    core_groups = []          # per core: {(dt,win): edge idx array}
    counts = np.zeros((n_cores, ndt, nwin), np.int64)
    for c in range(n_cores):
        sel = np.nonzero((row >= c * rows_pc) & (row < (c + 1) * rows_pc))[0]
        r_loc = row[sel] - c * rows_pc
        dt = r_loc // 128
        win = col[sel] // dims["win"]
        key = dt.astype(np.int64) * nwin + win
        order = np.argsort(key, kind="stable")
        sel, key = sel[order], key[order]
        uniq, starts = np.unique(key, return_index=True)
        starts = list(starts) + [len(sel)]
        groups = {}
        for i, k in enumerate(uniq):
            g_dt, g_w = int(k) // nwin, int(k) % nwin
            idxs = sel[starts[i]:starts[i + 1]]
            groups[(g_dt, g_w)] = idxs
            counts[c, g_dt, g_w] = len(idxs)
        core_groups.append(groups)

    B = -(-counts.max(axis=0) // 128)          # [ndt, nwin] blocks (>=0)
    B = np.maximum(B, (counts.max(axis=0) > 0).astype(np.int64))

    # chunk layout: chunks of `ch` dest-tiles; within a chunk blocks are
    # window-major: for w: for dt in chunk: B[dt][w] blocks.
    chunks = []
    block_off = 0
    for c0 in range(0, ndt, ch):
        dts = list(range(c0, min(c0 + ch, ndt)))
        woff, blk = [], 0
        for w in range(nwin):
            woff.append(blk)
            blk += int(B[dts, w].sum()) if len(dts) else 0
        # per (dt) block index lists
        dt_blocks = {}
        for dt in dts:
            lst = []
            for w in range(nwin):
                base = woff[w] + int(B[dts[0]:dt, w].sum())
                lst += list(range(base, base + int(B[dt, w])))
            dt_blocks[dt] = lst
        nb_win = [int(B[dts, w].sum()) for w in range(nwin)]
        chunks.append(dict(dts=dts, nb_win=nb_win, nbc=blk,
                           block_off=block_off, dt_blocks=dt_blocks, woff=woff))
        block_off += blk
    nb_tot = block_off

    meta = dict(B=B, chunks=chunks, nb_tot=nb_tot, ndt=ndt, nwin=nwin,
                rows_pc=rows_pc)
    return meta, core_groups


def fill_edge_arrays(meta, groups, row, col, ew, core, dims):
    """Build per-core (idx_wrapped[i16 128 x S/16], dest[f16 128 x NB],
    w[f16 128 x NB]) slot arrays in the chunk/window-major block layout."""
    nwin = meta["nwin"]
    rows_pc = meta["rows_pc"]
    nslots = meta["nb_tot"] * 128
    idx_f = np.zeros(nslots, np.int32)
    dest_f = np.zeros(nslots, np.float16)
    w_f = np.zeros(nslots, np.float16)
    for chk in meta["chunks"]:
        for dt in chk["dts"]:
            blocks = chk["dt_blocks"][dt]
            # window sub-ranges of this dt's block list
            bpos = 0
            for w in range(nwin):
                nb = int(meta["B"][dt, w])
                if nb == 0:
                    continue
                base_blk = blocks[bpos]
                bpos += nb
                base = (chk["block_off"] + base_blk) * 128
                e = groups.get((dt, w))
                if e is None:
                    idx_f[base:base + nb * 128] = 0  # window-local 0
                    continue
                cnt = len(e)
                idx_f[base:base + cnt] = col[e] - w * dims["win"]
                dest_f[base:base + cnt] = (row[e] - core * rows_pc - dt * 128
                                           ).astype(np.float16)
                w_f[base:base + cnt] = ew[e].astype(np.float16)
                # padding slots already 0 (window-local idx 0, w=0)
    assert (idx_f >= 0).all() and (idx_f < 32768).all()
    idx16 = idx_f.astype(np.int16).reshape(-1, 16).T          # [16, S/16]
    idx16 = np.tile(idx16, (8, 1))                            # [128, S/16]
    # edw[p, 0, b] = dest_slot, edw[p, 1, b] = weight (f32: DVE scalar APs)
    edw = np.empty((128, 2, meta["nb_tot"]), np.float32)
    edw[:, 0, :] = dest_f.astype(np.float32).reshape(-1, 128).T
    edw[:, 1, :] = w_f.astype(np.float32).reshape(-1, 128).T
    return idx16, edw


# ------------------------------------------------------------ kernel builder

def build_nc(meta, dims, debug=False):
    n = dims["n_nodes"]
    nfeat, nhid, ncls = dims["nfeat"], dims["nhid"], dims["nclass"]
    rows = meta["rows_pc"]
    nwin = meta["nwin"]
    win = dims["win"]
    nb_tot = meta["nb_tot"]
    nslots = nb_tot * 128
    ndt = meta["ndt"]
    nrt = -(-rows // 128)
    nkt = nfeat // 128
    t2w = 128  # layer-2 table width (nclass padded so gather elem = 256B)
    assert nhid == 128

    nc = bacc.Bacc("TRN2", target_bir_lowering=False, debug=debug)

    xT_d = nc.dram_tensor("xTk", [128, nkt, rows], F16, kind="ExternalInput")
    mask_d = nc.dram_tensor("maskT2", [nhid, rows], F16, kind="ExternalInput")
    w1_d = nc.dram_tensor("W1", [nfeat, nhid], F16, kind="ExternalInput")
    b1_d = nc.dram_tensor("b1row", [1, nhid], F16, kind="ExternalInput")
    w2_d = nc.dram_tensor("W2", [nhid, ncls], F16, kind="ExternalInput")
    b2_d = nc.dram_tensor("b2row", [1, ncls], F16, kind="ExternalInput")
    eidx_d = nc.dram_tensor("eidx", [128, nslots // 16], I16, kind="ExternalInput")
    edw_d = nc.dram_tensor("edw", [128, 2, nb_tot], F32, kind="ExternalInput")
    out_d = nc.dram_tensor("out", [rows, ncls], F32, kind="ExternalOutput")

    rg = [list(range(dims["n_cores"]))]

    with tile.TileContext(nc) as tc:
        with (
            tc.tile_pool(name="dram", bufs=1, space="DRAM") as dpool,
            tc.tile_pool(name="const", bufs=1) as cpool,
            tc.tile_pool(name="persist", bufs=1) as hpool,
            tc.tile_pool(name="stream", bufs=3) as spool,
            tc.tile_pool(name="chunkio", bufs=2) as iopool,
            tc.tile_pool(name="big", bufs=2) as bigpool,
            tc.tile_pool(name="psum", bufs=4, space="PSUM") as ppool,
            tc.tile_pool(name="psum2", bufs=2, space="PSUM") as ppool2,
        ):
            nc.gpsimd.load_library(mlp)

            sa = dims["split_a"]
            nca = dims["n_cores"] * sa
            assert sa % 128 == 0 and nca % win == 0
            sup1_ba = dpool.tile([sa, nhid], F16, name="sup1_ba")
            sup1_bb = dpool.tile([rows - sa, nhid], F16, name="sup1_bb")
            table1a = dpool.tile([nca, nhid], F16, addr_space="Shared",
                                 name="table1a")
            table1b = dpool.tile([n - nca, nhid], F16, addr_space="Shared",
                                 name="table1b")
            sup2_ba = dpool.tile([sa, t2w], F16, name="sup2_ba")
            sup2_bb = dpool.tile([rows - sa, t2w], F16, name="sup2_bb")
            table2a = dpool.tile([nca, t2w], F16, addr_space="Shared",
                                 name="table2a")
            table2b = dpool.tile([n - nca, t2w], F16, addr_space="Shared",
                                 name="table2b")

            # constants
            w1_t = cpool.tile([128, nkt, nhid], F16, name="w1kt")
            for kt in range(nkt):
                nc.sync.dma_start(w1_t[:, kt, :], w1_d[kt * 128:(kt + 1) * 128, :])
            b1_t = cpool.tile([1, nhid], F16, name="b1_t")
            nc.sync.dma_start(b1_t[:], b1_d[:, :])
            w2_t = cpool.tile([128, ncls], F16, name="w2_t")
            nc.sync.dma_start(w2_t[:], w2_d[:, :])
            b2_t = cpool.tile([1, ncls], F16, name="b2_t")
            nc.sync.dma_start(b2_t[:], b2_d[:, :])
            ones_t = cpool.tile([1, 128], F16, name="ones_t")
            nc.vector.memset(ones_t[:], 1.0)
            iota_t = cpool.tile([128, 128], F16, name="iota_t")
            nc.gpsimd.iota(iota_t[:], pattern=[[1, 128]], base=0,
                           channel_multiplier=0,
                           allow_small_or_imprecise_dtypes=True)
            hT = hpool.tile([nhid, rows], F16, name="hT")

            # ---------------- GEMM1: support1 = x @ W1 + b1 -> table1 shard
            for rt in range(nrt):
                r0 = rt * 128
                rc = min(128, rows - r0)
                xk = spool.tile([128, nkt, 128], F16, name="xk")
                nc.sync.dma_start(xk[:, :, :rc], xT_d[:, :, r0:r0 + rc])
                ps = ppool.tile([128, nhid], F32, name="ps", tag="ps")
                nc.tensor.matmul(ps[:rc, :], ones_t[:1, :rc], b1_t[:1, :],
                                 start=True, stop=False)
                for kt in range(nkt):
                    nc.tensor.matmul(ps[:rc, :], xk[:, kt, :rc], w1_t[:, kt, :],
                                     start=False, stop=(kt == nkt - 1))
                sup = spool.tile([128, nhid], F16, name="sup")
                nc.scalar.activation(sup[:rc, :], ps[:rc, :],
                                     mybir.ActivationFunctionType.Copy)
                if r0 < sa:
                    nc.sync.dma_start(sup1_ba[r0:r0 + rc, :], sup[:rc, :])
                else:
                    nc.sync.dma_start(sup1_bb[r0 - sa:r0 - sa + rc, :],
                                      sup[:rc, :])

            nc.gpsimd.collective_compute(
                "AllGather", mybir.AluOpType.bypass, replica_groups=rg,
                ins=[sup1_ba[:].opt()], outs=[table1a[:].opt()])
            nc.gpsimd.collective_compute(
                "AllGather", mybir.AluOpType.bypass, replica_groups=rg,
                ins=[sup1_bb[:].opt()], outs=[table1b[:].opt()])

            # ---------------- layer loops (shared structure)
            def edge_layer(table_a, table_b, layer):
                ag2a_done = [False]
                for chk in meta["chunks"]:
                    nbc = chk["nbc"]
                    if nbc == 0:
                        continue
                    boff = chk["block_off"]
                    msgs = bigpool.tile([128, nbc, 128], F16, name="msgs",
                                        tag="msgs")
                    s_t = bigpool.tile([128, nbc, 128], F16, name="s_t",
                                       tag="s_t")
                    idx_t = iopool.tile([128, nbc * 8], I16, name="idx_t",
                                        tag="idx_t")
                    edw_t = iopool.tile([128, 2, nbc], F32, name="edw_t",
                                        tag="edw_t")
                    nc.sync.dma_start(idx_t[:],
                                      eidx_d[:, boff * 8:(boff + nbc) * 8])
                    nc.sync.dma_start(edw_t[:], edw_d[:, :, boff:boff + nbc])

                    wb = 0
                    for w in range(nwin):
                        nbw = chk["nb_win"][w]
                        lo = w * win
                        hi = min(n, lo + win)
                        if hi <= nca:
                            wsrc = table_a[lo:hi, :]
                        else:
                            assert lo >= nca
                            wsrc = table_b[lo - nca:hi - nca, :]
                        for g0 in range(0, nbw, 8):
                            gb = min(8, nbw - g0)
                            nc.gpsimd.dma_gather(
                                msgs[:, wb + g0:wb + g0 + gb, :],
                                wsrc,
                                idx_t[:, (wb + g0) * 8:(wb + g0 + gb) * 8],
                                gb * 128, gb * 128, 128)
                        wb += nbw

                    for b in range(nbc):
                        nc.vector.tensor_scalar(
                            s_t[:, b, :], iota_t[:],
                            edw_t[:, 0, b:b + 1], edw_t[:, 1, b:b + 1],
                            mybir.AluOpType.is_equal, mybir.AluOpType.mult)

                    if layer == 1:
                        mk = iopool.tile([128, len(chk["dts"]) * 128], F16,
                                         name="mk", tag="mk")
                        m0 = chk["dts"][0] * 128
                        mw = min(rows, (chk["dts"][-1] + 1) * 128) - m0
                        nc.sync.dma_start(mk[:, :mw], mask_d[:, m0:m0 + mw])

                    for di, dt in enumerate(chk["dts"]):
                        blocks = chk["dt_blocks"][dt]
                        if not blocks:
                            continue
                        d0 = dt * 128
                        dc = min(128, rows - d0)
                        if layer == 1:
                            ph = ppool.tile([128, 128], F32, name="ph", tag="ps")
                            for i, b in enumerate(blocks):
                                nc.tensor.matmul(ph[:, :dc], msgs[:, b, :],
                                                 s_t[:, b, :dc],
                                                 start=(i == 0),
                                                 stop=(i == len(blocks) - 1))
                            nc.vector.scalar_tensor_tensor(
                                hT[:, d0:d0 + dc], ph[:, :dc], 0.0,
                                mk[:, di * 128:di * 128 + dc],
                                op0=mybir.AluOpType.max,
                                op1=mybir.AluOpType.mult)
                            # GEMM2 for this row-tile immediately (hides the
                            # layer-1 -> AllGather2 join under the L1 gathers)
                            ps2 = ppool2.tile([128, ncls], F32, name="ps2",
                                              tag="ps2")
                            nc.tensor.matmul(ps2[:dc, :], ones_t[:1, :dc],
                                             b2_t[:1, :], start=True, stop=False)
                            nc.tensor.matmul(ps2[:dc, :], hT[:, d0:d0 + dc],
                                             w2_t[:, :], start=False, stop=True)
                            stg = spool.tile([128, t2w], F16, name="stg")
                            nc.vector.memset(stg[:, ncls:], 0.0)
                            nc.scalar.activation(
                                stg[:dc, :ncls], ps2[:dc, :],
                                mybir.ActivationFunctionType.Copy)
                            if d0 < sa:
                                nc.sync.dma_start(sup2_ba[d0:d0 + dc, :],
                                                  stg[:dc, :])
                            else:
                                nc.sync.dma_start(
                                    sup2_bb[d0 - sa:d0 - sa + dc, :],
                                    stg[:dc, :])
                        else:
                            po = ppool.tile([128, ncls], F32, name="po", tag="ps")
                            for i, b in enumerate(blocks):
                                nc.tensor.matmul(po[:dc, :], s_t[:, b, :dc],
                                                 msgs[:, b, :ncls],
                                                 start=(i == 0),
                                                 stop=(i == len(blocks) - 1))
                            ot = spool.tile([128, ncls], F32, name="ot")
                            nc.scalar.activation(
                                ot[:dc, :], po[:dc, :],
                                mybir.ActivationFunctionType.Copy)
                            nc.sync.dma_start(out_d[d0:d0 + dc, :], ot[:dc, :])
                    if (layer == 1 and not ag2a_done[0]
                            and (chk["dts"][-1] + 1) * 128 >= sa):
                        ag2a_done[0] = True
                        nc.gpsimd.collective_compute(
                            "AllGather", mybir.AluOpType.bypass,
                            replica_groups=rg,
                            ins=[sup2_ba[:].opt()], outs=[table2a[:].opt()])

            edge_layer(table1a, table1b, 1)

            nc.gpsimd.collective_compute(
                "AllGather", mybir.AluOpType.bypass, replica_groups=rg,
                ins=[sup2_bb[:].opt()], outs=[table2b[:].opt()])

            edge_layer(table2a, table2b, 2)

    nc.compile()
    return nc


# ----------------------------------------------------------------- run logic

_CACHE = {}


def _get_mask2(n, nhid, dropout):
    key = (n, nhid, dropout)
    if key not in _CACHE:
        import jax
        with jax.default_device(jax.devices("cpu")[0]):
            keep = 1.0 - dropout
            m = jax.random.bernoulli(jax.random.key(42), keep, (n, nhid))
            _CACHE[key] = (np.asarray(m).astype(np.float32) / keep)
    return _CACHE[key]


def remap_src(col, dims):
    """Source ids remapped to the split-AllGather table layout: first the
    A-halves (rows [0:split_a) of every core), then the B-halves."""
    rows = dims["n_nodes"] // dims["n_cores"]
    sa = dims["split_a"]
    c = col // rows
    r = col % rows
    return np.where(r < sa, c * sa + r,
                    dims["n_cores"] * sa + c * (rows - sa) + (r - sa)
                    ).astype(np.int32)


def prepare(inputs, dims):
    x = inputs["x"]; ew = inputs["edge_weight"]
    W1 = inputs["W1"]; b1 = inputs["b1"]; W2 = inputs["W2"]; b2 = inputs["b2"]
    row = inputs["row"]; col = remap_src(inputs["col"], dims)
    n_cores = dims["n_cores"]
    meta, groups = edge_structure(row, col, dims)
    rows = meta["rows_pc"]
    mask2 = _get_mask2(dims["n_nodes"], dims["nhid"], dims["dropout"])
    in_maps = []
    for c in range(n_cores):
        idx16, edw = fill_edge_arrays(meta, groups[c], row, col, ew, c, dims)
        xT = x[c * rows:(c + 1) * rows].T.astype(np.float16)   # [nfeat, rows]
        nkt = dims["nfeat"] // 128
        xTk = np.ascontiguousarray(
            xT.reshape(nkt, 128, rows).transpose(1, 0, 2))     # [128, nkt, rows]
        in_maps.append(dict(
            xTk=xTk,
            maskT2=np.ascontiguousarray(mask2[c * rows:(c + 1) * rows].T
                                        ).astype(np.float16),
            W1=W1.astype(np.float16),
            b1row=b1.reshape(1, -1).astype(np.float16),
            W2=W2.astype(np.float16),
            b2row=b2.reshape(1, -1).astype(np.float16),
            eidx=idx16, edw=edw,
        ))
    return meta, in_maps


def run(inputs, dims, trace=False):
    meta, in_maps = prepare(inputs, dims)
    key = ("nc", tuple(meta["B"].ravel()), tuple(sorted(dims.items())))
    if key not in _CACHE:
        _CACHE[key] = build_nc(meta, dims)
    nc = _CACHE[key]
    res = run_bass_kernel_spmd(nc, in_maps,
                               core_ids=list(range(dims["n_cores"])),
                               trace=trace)
    out = np.concatenate([r["out"] for r in res.results], axis=0)
    return out, res


def kernel(**inputs):
    out, _ = run(inputs, REAL_DIMS)
    return out


# revision 17
# speedup vs baseline: 1.1226x; 1.0097x over previous
"""Distributed GCN backbone kernel for Trainium2 (8 NeuronCores).

Strategy (per sharding hint): nodes sharded by rows across 8 cores; edges
partitioned by destination node so the segment-sum stays local; the support
table (h @ W) is all-gathered across cores each layer (halo = full table
since sources are uniform-random); per-edge features fetched with
dma_gather; segment-sum realized as PE matmuls against on-the-fly one-hot
selector matrices S[e,d] = (iota==dest_slot)*w built on the vector engine.

Dataflow per core (ROWS = N/8 local destination rows):
  GEMM1: support1 = x_shard @ W1 + b1            (PE, fp16)
  AllGather -> table1 [N, 128] fp16 (Shared DRAM)
  L1: per chunk of dest-tiles: dma_gather msgs from table1 (4 int16 windows),
      build S, psum_hT[f,d] += msgs_blk^T @ S_blk, then
      hT = relu(psum) * (dropout_mask*2)          (DVE scalar_tensor_tensor)
  GEMM2: support2 = hT^T @ W2 + b2 -> padded to 128 cols
  AllGather -> table2 [N, 128] fp16
  L2: gather msgs2 from table2, rebuild S, psum_out[d,c] += S_blk^T @ msgs2_blk
      -> out [ROWS, 64] fp32

kernel(**inputs) takes the full unsharded inputs and returns the full
[100000, 64] fp32 output.
"""

import numpy as np

import concourse.bacc as bacc
import concourse.bass as bass
import concourse.mybir as mybir
import concourse.tile as tile
from concourse.bass_utils import run_bass_kernel_spmd
from concourse.library_config import mlp

F16 = mybir.dt.float16
F32 = mybir.dt.float32
I16 = mybir.dt.int16

REAL_DIMS = dict(
    n_nodes=100000, nfeat=512, nhid=128, nclass=64,
    n_cores=8, win=32768, ch_dt=6, dropout=0.5, split_a=4096,
)


# ----------------------------------------------------------------- host prep

def edge_structure(row, col, dims):
    """Group edges per core by (dest_tile, src_window), pad each group to a
    multiple of 128 (block) with (src=window_base, dest=0, w=0) slots.
    Block counts per (dt, win) are the max over cores so one NEFF serves all
    cores. Returns (meta, per_core_groups) where per_core_groups[c] is a list
    of (dt, win, slot_base, edge_indices) fill directives.
    """
    n_cores = dims["n_cores"]
    n = dims["n_nodes"]
    rows_pc = n // n_cores
    ndt = -(-rows_pc // 128)
    nwin = -(-n // dims["win"])
    ch = dims["ch_dt"]

    core_groups = []          # per core: {(dt,win): edge idx array}
    counts = np.zeros((n_cores, ndt, nwin), np.int64)
    for c in range(n_cores):
        sel = np.nonzero((row >= c * rows_pc) & (row < (c + 1) * rows_pc))[0]
        r_loc = row[sel] - c * rows_pc
        dt = r_loc // 128
        win = col[sel] // dims["win"]
        key = dt.astype(np.int64) * nwin + win
        order = np.argsort(key, kind="stable")
        sel, key = sel[order], key[order]
        uniq, starts = np.unique(key, return_index=True)
        starts = list(starts) + [len(sel)]
        groups = {}
        for i, k in enumerate(uniq):
            g_dt, g_w = int(k) // nwin, int(k) % nwin
            idxs = sel[starts[i]:starts[i + 1]]
            groups[(g_dt, g_w)] = idxs
            counts[c, g_dt, g_w] = len(idxs)
        core_groups.append(groups)

    B = -(-counts.max(axis=0) // 128)          # [ndt, nwin] blocks (>=0)
    B = np.maximum(B, (counts.max(axis=0) > 0).astype(np.int64))

    # chunk layout: chunks of `ch` dest-tiles; within a chunk blocks are
    # window-major: for w: for dt in chunk: B[dt][w] blocks.
    chunks = []
    block_off = 0
    for c0 in range(0, ndt, ch):
        dts = list(range(c0, min(c0 + ch, ndt)))
        woff, blk = [], 0
        for w in range(nwin):
            woff.append(blk)
            blk += int(B[dts, w].sum()) if len(dts) else 0
        # per (dt) block index lists
        dt_blocks = {}
        for dt in dts:
            lst = []
            for w in range(nwin):
                base = woff[w] + int(B[dts[0]:dt, w].sum())
                lst += list(range(base, base + int(B[dt, w])))
            dt_blocks[dt] = lst
        nb_win = [int(B[dts, w].sum()) for w in range(nwin)]
        chunks.append(dict(dts=dts, nb_win=nb_win, nbc=blk,
                           block_off=block_off, dt_blocks=dt_blocks, woff=woff))
        block_off += blk
    nb_tot = block_off

    meta = dict(B=B, chunks=chunks, nb_tot=nb_tot, ndt=ndt, nwin=nwin,
                rows_pc=rows_pc)
    return meta, core_groups


def fill_edge_arrays(meta, groups, row, col, ew, core, dims):
    """Build per-core (idx_wrapped[i16 128 x S/16], dest[f16 128 x NB],
    w[f16 128 x NB]) slot arrays in the chunk/window-major block layout."""
    nwin = meta["nwin"]
    rows_pc = meta["rows_pc"]
    nslots = meta["nb_tot"] * 128
    idx_f = np.zeros(nslots, np.int32)
    dest_f = np.zeros(nslots, np.float16)
    w_f = np.zeros(nslots, np.float16)
    for chk in meta["chunks"]:
        for dt in chk["dts"]:
            blocks = chk["dt_blocks"][dt]
            # window sub-ranges of this dt's block list
            bpos = 0
            for w in range(nwin):
                nb = int(meta["B"][dt, w])
                if nb == 0:
                    continue
                base_blk = blocks[bpos]
                bpos += nb
                base = (chk["block_off"] + base_blk) * 128
                e = groups.get((dt, w))
                if e is None:
                    idx_f[base:base + nb * 128] = 0  # window-local 0
                    continue
                cnt = len(e)
                idx_f[base:base + cnt] = col[e] - w * dims["win"]
                dest_f[base:base + cnt] = (row[e] - core * rows_pc - dt * 128
                                           ).astype(np.float16)
                w_f[base:base + cnt] = ew[e].astype(np.float16)
                # padding slots already 0 (window-local idx 0, w=0)
    assert (idx_f >= 0).all() and (idx_f < 32768).all()
    idx16 = idx_f.astype(np.int16).reshape(-1, 16).T          # [16, S/16]
    idx16 = np.tile(idx16, (8, 1))                            # [128, S/16]
    # edw[p, 0, b] = dest_slot, edw[p, 1, b] = weight (f32: DVE scalar APs)
    edw = np.empty((128, 2, meta["nb_tot"]), np.float32)
    edw[:, 0, :] = dest_f.astype(np.float32).reshape(-1, 128).T
    edw[:, 1, :] = w_f.astype(np.float32).reshape(-1, 128).T
    return idx16, edw


# ------------------------------------------------------------ kernel builder

def build_nc(meta, dims, debug=False):
    n = dims["n_nodes"]
    nfeat, nhid, ncls = dims["nfeat"], dims["nhid"], dims["nclass"]
    rows = meta["rows_pc"]
    nwin = meta["nwin"]
    win = dims["win"]
    nb_tot = meta["nb_tot"]
    nslots = nb_tot * 128
    ndt = meta["ndt"]
    nrt = -(-rows // 128)
    nkt = nfeat // 128
    t2w = 128  # layer-2 table width (nclass padded so gather elem = 256B)
    assert nhid == 128

    nc = bacc.Bacc("TRN2", target_bir_lowering=False, debug=debug)

    xT_d = nc.dram_tensor("xTk", [128, nkt, rows], F16, kind="ExternalInput")
    mask_d = nc.dram_tensor("maskT2", [nhid, rows], F16, kind="ExternalInput")
    w1_d = nc.dram_tensor("W1", [nfeat, nhid], F16, kind="ExternalInput")
    b1_d = nc.dram_tensor("b1row", [1, nhid], F16, kind="ExternalInput")
    w2_d = nc.dram_tensor("W2", [nhid, ncls], F16, kind="ExternalInput")
    b2_d = nc.dram_tensor("b2row", [1, ncls], F16, kind="ExternalInput")
    eidx_d = nc.dram_tensor("eidx", [128, nslots // 16], I16, kind="ExternalInput")
    edw_d = nc.dram_tensor("edw", [128, 2, nb_tot], F32, kind="ExternalInput")
    out_d = nc.dram_tensor("out", [rows, ncls], F32, kind="ExternalOutput")

    rg = [list(range(dims["n_cores"]))]

    with tile.TileContext(nc) as tc:
        with (
            tc.tile_pool(name="dram", bufs=1, space="DRAM") as dpool,
            tc.tile_pool(name="const", bufs=1) as cpool,
            tc.tile_pool(name="persist", bufs=1) as hpool,
            tc.tile_pool(name="stream", bufs=3) as spool,
            tc.tile_pool(name="chunkio", bufs=2) as iopool,
            tc.tile_pool(name="big", bufs=2) as bigpool,
            tc.tile_pool(name="psum", bufs=4, space="PSUM") as ppool,
            tc.tile_pool(name="psum2", bufs=2, space="PSUM") as ppool2,
        ):
            nc.gpsimd.load_library(mlp)

            segpc = win // dims["n_cores"]
            seg_rows = [segpc] * (nwin - 1) + [rows - (nwin - 1) * segpc]
            seg_lo = [s * segpc for s in range(nwin)]
            tab_rows = [sr * dims["n_cores"] for sr in seg_rows]
            sup1_s = [dpool.tile([seg_rows[s], nhid], F16, name=f"sup1_s{s}")
                      for s in range(nwin)]
            tab1_s = [dpool.tile([tab_rows[s], nhid], F16, addr_space="Shared",
                                 name=f"tab1_s{s}") for s in range(nwin)]
            sup2_s = [dpool.tile([seg_rows[s], t2w], F16, name=f"sup2_s{s}")
                      for s in range(nwin)]
            tab2_s = [dpool.tile([tab_rows[s], t2w], F16, addr_space="Shared",
                                 name=f"tab2_s{s}") for s in range(nwin)]

            def seg_write(bounces, r0, rc, tile_ap):
                # write [r0, r0+rc) rows of the per-core bounce, split across
                # whichever window-segments it overlaps
                done = 0
                while done < rc:
                    s = min((r0 + done) // segpc, nwin - 1)
                    off = r0 + done - seg_lo[s]
                    take = min(rc - done, seg_rows[s] - off)
                    nc.sync.dma_start(bounces[s][off:off + take, :],
                                      tile_ap[done:done + take, :])
                    done += take

            # constants
            w1_t = cpool.tile([128, nkt, nhid], F16, name="w1kt")
            for kt in range(nkt):
                nc.sync.dma_start(w1_t[:, kt, :], w1_d[kt * 128:(kt + 1) * 128, :])
            b1_t = cpool.tile([1, nhid], F16, name="b1_t")
            nc.sync.dma_start(b1_t[:], b1_d[:, :])
            w2_t = cpool.tile([128, ncls], F16, name="w2_t")
            nc.sync.dma_start(w2_t[:], w2_d[:, :])
            b2_t = cpool.tile([1, ncls], F16, name="b2_t")
            nc.sync.dma_start(b2_t[:], b2_d[:, :])
            ones_t = cpool.tile([1, 128], F16, name="ones_t")
            nc.vector.memset(ones_t[:], 1.0)
            iota_t = cpool.tile([128, 128], F16, name="iota_t")
            nc.gpsimd.iota(iota_t[:], pattern=[[1, 128]], base=0,
                           channel_multiplier=0,
                           allow_small_or_imprecise_dtypes=True)
            hT = hpool.tile([nhid, rows], F16, name="hT")

            # ---------------- GEMM1: support1 = x @ W1 + b1 -> table1 shard
            for rt in range(nrt):
                r0 = rt * 128
                rc = min(128, rows - r0)
                xk = spool.tile([128, nkt, 128], F16, name="xk")
                nc.sync.dma_start(xk[:, :, :rc], xT_d[:, :, r0:r0 + rc])
                ps = ppool.tile([128, nhid], F32, name="ps", tag="ps")
                nc.tensor.matmul(ps[:rc, :], ones_t[:1, :rc], b1_t[:1, :],
                                 start=True, stop=False)
                for kt in range(nkt):
                    nc.tensor.matmul(ps[:rc, :], xk[:, kt, :rc], w1_t[:, kt, :],
                                     start=False, stop=(kt == nkt - 1))
                sup = spool.tile([128, nhid], F16, name="sup")
                nc.scalar.activation(sup[:rc, :], ps[:rc, :],
                                     mybir.ActivationFunctionType.Copy)
                seg_write(sup1_s, r0, rc, sup)

            for s in range(nwin):
                nc.gpsimd.collective_compute(
                    "AllGather", mybir.AluOpType.bypass, replica_groups=rg,
                    ins=[sup1_s[s][:].opt()], outs=[tab1_s[s][:].opt()])

            # ---------------- layer loops (shared structure)
            def edge_layer(tabs, sup2_tabs, layer):
                ag2_next = [0]
                for chk in meta["chunks"]:
                    nbc = chk["nbc"]
                    if nbc == 0:
                        continue
                    boff = chk["block_off"]
                    msgs = bigpool.tile([128, nbc, 128], F16, name="msgs",
                                        tag="msgs")
                    s_t = bigpool.tile([128, nbc, 128], F16, name="s_t",
                                       tag="s_t")
                    idx_t = iopool.tile([128, nbc * 8], I16, name="idx_t",
                                        tag="idx_t")
                    edw_t = iopool.tile([128, 2, nbc], F32, name="edw_t",
                                        tag="edw_t")
                    nc.sync.dma_start(idx_t[:],
                                      eidx_d[:, boff * 8:(boff + nbc) * 8])
                    nc.sync.dma_start(edw_t[:], edw_d[:, :, boff:boff + nbc])

                    wb = 0
                    for w in range(nwin):
                        nbw = chk["nb_win"][w]
                        wsrc = tabs[w][:, :]
                        for g0 in range(0, nbw, 8):
                            gb = min(8, nbw - g0)
                            nc.gpsimd.dma_gather(
                                msgs[:, wb + g0:wb + g0 + gb, :],
                                wsrc,
                                idx_t[:, (wb + g0) * 8:(wb + g0 + gb) * 8],
                                gb * 128, gb * 128, 128)
                        wb += nbw

                    for b in range(nbc):
                        nc.vector.tensor_scalar(
                            s_t[:, b, :], iota_t[:],
                            edw_t[:, 0, b:b + 1], edw_t[:, 1, b:b + 1],
                            mybir.AluOpType.is_equal, mybir.AluOpType.mult)

                    if layer == 1:
                        mk = iopool.tile([128, len(chk["dts"]) * 128], F16,
                                         name="mk", tag="mk")
                        m0 = chk["dts"][0] * 128
                        mw = min(rows, (chk["dts"][-1] + 1) * 128) - m0
                        nc.sync.dma_start(mk[:, :mw], mask_d[:, m0:m0 + mw])

                    for di, dt in enumerate(chk["dts"]):
                        blocks = chk["dt_blocks"][dt]
                        if not blocks:
                            continue
                        d0 = dt * 128
                        dc = min(128, rows - d0)
                        if layer == 1:
                            ph = ppool.tile([128, 128], F32, name="ph", tag="ps")
                            for i, b in enumerate(blocks):
                                nc.tensor.matmul(ph[:, :dc], msgs[:, b, :],
                                                 s_t[:, b, :dc],
                                                 start=(i == 0),
                                                 stop=(i == len(blocks) - 1))
                            nc.vector.scalar_tensor_tensor(
                                hT[:, d0:d0 + dc], ph[:, :dc], 0.0,
                                mk[:, di * 128:di * 128 + dc],
                                op0=mybir.AluOpType.max,
                                op1=mybir.AluOpType.mult)
                            # GEMM2 for this row-tile immediately (hides the
                            # layer-1 -> AllGather2 join under the L1 gathers)
                            ps2 = ppool2.tile([128, ncls], F32, name="ps2",
                                              tag="ps2")
                            nc.tensor.matmul(ps2[:dc, :], ones_t[:1, :dc],
                                             b2_t[:1, :], start=True, stop=False)
                            nc.tensor.matmul(ps2[:dc, :], hT[:, d0:d0 + dc],
                                             w2_t[:, :], start=False, stop=True)
                            stg = spool.tile([128, t2w], F16, name="stg")
                            nc.vector.memset(stg[:, ncls:], 0.0)
                            nc.scalar.activation(
                                stg[:dc, :ncls], ps2[:dc, :],
                                mybir.ActivationFunctionType.Copy)
                            seg_write(sup2_s, d0, dc, stg)
                        else:
                            po = ppool.tile([128, ncls], F32, name="po", tag="ps")
                            for i, b in enumerate(blocks):
                                nc.tensor.matmul(po[:dc, :], s_t[:, b, :dc],
                                                 msgs[:, b, :ncls],
                                                 start=(i == 0),
                                                 stop=(i == len(blocks) - 1))
                            ot = spool.tile([128, ncls], F32, name="ot")
                            nc.scalar.activation(
                                ot[:dc, :], po[:dc, :],
                                mybir.ActivationFunctionType.Copy)
                            nc.sync.dma_start(out_d[d0:d0 + dc, :], ot[:dc, :])
                    if layer == 1:
                        done_rows = min((chk["dts"][-1] + 1) * 128, rows)
                        while (ag2_next[0] < nwin
                               and done_rows >= seg_lo[ag2_next[0]]
                               + seg_rows[ag2_next[0]]):
                            s = ag2_next[0]
                            ag2_next[0] += 1
                            nc.gpsimd.collective_compute(
                                "AllGather", mybir.AluOpType.bypass,
                                replica_groups=rg,
                                ins=[sup2_s[s][:].opt()],
                                outs=[sup2_tabs[s][:].opt()])

            edge_layer(tab1_s, tab2_s, 1)
            edge_layer(tab2_s, None, 2)

    nc.compile()
    return nc


# ----------------------------------------------------------------- run logic

_CACHE = {}


def _get_mask2(n, nhid, dropout):
    key = (n, nhid, dropout)
    if key not in _CACHE:
        import jax
        with jax.default_device(jax.devices("cpu")[0]):
            keep = 1.0 - dropout
            m = jax.random.bernoulli(jax.random.key(42), keep, (n, nhid))
            _CACHE[key] = (np.asarray(m).astype(np.float32) / keep)
    return _CACHE[key]


def remap_src(col, dims):
    """Source ids remapped so each gather window w equals the concat of every
    core's segment-w rows (per-core segment = win//n_cores rows, last ragged).
    Lets one AllGather per window unblock that window's gathers early."""
    ncores = dims["n_cores"]
    rows = dims["n_nodes"] // ncores
    segpc = dims["win"] // ncores
    nseg = -(-dims["n_nodes"] // dims["win"])
    c = col // rows
    r = col % rows
    s = np.minimum(r // segpc, nseg - 1)
    seg_rows = np.where(s < nseg - 1, segpc, rows - (nseg - 1) * segpc)
    return (s * dims["win"] + c * seg_rows + (r - s * segpc)).astype(np.int32)


def prepare(inputs, dims):
    x = inputs["x"]; ew = inputs["edge_weight"]
    W1 = inputs["W1"]; b1 = inputs["b1"]; W2 = inputs["W2"]; b2 = inputs["b2"]
    row = inputs["row"]; col = remap_src(inputs["col"], dims)
    n_cores = dims["n_cores"]
    meta, groups = edge_structure(row, col, dims)
    rows = meta["rows_pc"]
    mask2 = _get_mask2(dims["n_nodes"], dims["nhid"], dims["dropout"])
    in_maps = []
    for c in range(n_cores):
        idx16, edw = fill_edge_arrays(meta, groups[c], row, col, ew, c, dims)
        xT = x[c * rows:(c + 1) * rows].T.astype(np.float16)   # [nfeat, rows]
        nkt = dims["nfeat"] // 128
        xTk = np.ascontiguousarray(
            xT.reshape(nkt, 128, rows).transpose(1, 0, 2))     # [128, nkt, rows]
        in_maps.append(dict(
            xTk=xTk,
            maskT2=np.ascontiguousarray(mask2[c * rows:(c + 1) * rows].T
                                        ).astype(np.float16),
            W1=W1.astype(np.float16),
            b1row=b1.reshape(1, -1).astype(np.float16),
            W2=W2.astype(np.float16),
            b2row=b2.reshape(1, -1).astype(np.float16),
            eidx=idx16, edw=edw,
        ))
    return meta, in_maps


def run(inputs, dims, trace=False):
    meta, in_maps = prepare(inputs, dims)
    key = ("nc", tuple(meta["B"].ravel()), tuple(sorted(dims.items())))
    if key not in _CACHE:
        _CACHE[key] = build_nc(meta, dims)
    nc = _CACHE[key]
    res = run_bass_kernel_spmd(nc, in_maps,
                               core_ids=list(range(dims["n_cores"])),
                               trace=trace)
    out = np.concatenate([r["out"] for r in res.results], axis=0)
    return out, res


def kernel(**inputs):
    out, _ = run(inputs, REAL_DIMS)
    return out
